# revision 1
# baseline (speedup 1.0000x reference)
"""Trainium2 Bass kernel for nn_GawKAN (2-layer graph-wavelet net with 4
global-MSE sigmoid mix gates).

Strategy: data-parallel over batch B=128 across 8 NeuronCores (16 batches per
core); adj + weights replicated. The network is algebraically collapsed on the
host into a chain over difference tensors d0..d3 (see host algebra below);
each theta_i = sigmoid(mean(d_i^2)) needs one 8-core AllReduce of a [128,1]
partial-sum vector. All bias constants are folded into an augmented ones-row
of x, so every PSUM evacuation is a plain copy / fused scalar_tensor_tensor.

Chain (per core, all layout-F [feature partition, (batch, node) free]):
  d0  = adjx@Wg0 - x@W_rg0x (+c)         adjx = adj@x
  d1  = x@W_A1 + d0@(th0*M0I) (+c)
  xw' = x@W_rw0x + th1*d1 (+c)
  d2  = (adj@xw')@Wg1 - x@W_rg1x + xw' (+c)
  d3  = x@W_A3 + d2@(th2*M1) (+c)
  out = x@W_rw1x + th3*d3 (+c)

dtypes: x / x-side folded weights fp32 (consumed as float32r by the PE);
activation tensors d*, xw', adjx, adjw and adj/graph weights bf16; PSUM fp32.
"""
import numpy as np
import ml_dtypes

import concourse.bass as bass
import concourse.bacc as bacc
import concourse.tile as tile
import concourse.mybir as mybir
from concourse import bass_isa
from concourse.bass_utils import run_bass_kernel_spmd

NCORES = 8
B, N, DIN, H, DOUT = 128, 512, 64, 256, 64
BL = B // NCORES                 # 16 batches per core
FREE = BL * N                    # 8192 free positions per core
MAX_LEVEL, FULL_LEN = 3, 8

F32 = mybir.dt.float32
F32R = mybir.dt.float32r
BF16 = mybir.dt.bfloat16
AF = mybir.ActivationFunctionType
OP = mybir.AluOpType

CNT_H = float(B * N * H)         # mean() denominators (global, hence the AR)
CNT_O = float(B * N * DOUT)

_COMPILED = None


# ---------------------------------------------------------------- host algebra
def _make_patterns():
    pats = []
    for j in range(1, MAX_LEVEL + 1):
        vec = [2.0 ** (-j / 2.0)]
        for k in range(1, j + 1):
            seg = np.zeros(2 ** (j - k))
            seg[0] = 2.0 ** (-k / 2.0)
            vec.extend(seg.tolist())
        full = np.zeros(FULL_LEN, dtype=np.float64)
        arr = np.asarray(vec, dtype=np.float64)
        full[: arr.shape[0]] = arr
        pats.append(full)
    return np.stack(pats)


def _gumbel_coeff(logits, gumb, Wnl):
    P = _make_patterns()
    a = Wnl.astype(np.float64) @ P.T
    z = logits[None, :].astype(np.float64) + gumb.astype(np.float64)
    idx = np.argmax(z, axis=-1)
    return a[np.arange(a.shape[0]), idx]


def _fold_weights(inp):
    f = {k: np.asarray(v, np.float64) for k, v in inp.items()}
    coeff0 = _gumbel_coeff(f['logits0'], f['gumb0'], f['Wnl0'])
    coeff1 = _gumbel_coeff(f['logits1'], f['gumb1'], f['Wnl1'])

    M0I = coeff0[:, None] * f['Wwo0'] + np.eye(H)
    bwo0e = f['bnl0'] @ f['Wwo0'] + f['bwo0']
    M1 = coeff1[:, None] * f['Wwo1']
    bM1 = f['bnl1'] @ f['Wwo1'] + f['bwo1']

    W_rg0x = f['Wres'] @ f['Wrg0']; c_rg0x = f['bres'] @ f['Wrg0'] + f['brg0']
    W_rw0x = f['Wres'] @ f['Wrw0']; c_rw0x = f['bres'] @ f['Wrw0'] + f['brw0']
    W_rg1x = f['Wres'] @ f['Wrg1']; c_rg1x = f['bres'] @ f['Wrg1'] + f['brg1']
    W_rw1x = f['Wres'] @ f['Wrw1']; c_rw1x = f['bres'] @ f['Wrw1'] + f['brw1']

    c_d0 = f['bg0'] - c_rg0x
    W_A1 = W_rg0x @ M0I - W_rw0x
    c_d1 = c_rg0x @ M0I + bwo0e - c_rw0x
    c_d2 = f['bg1'] - c_rg1x
    W_A3 = W_rg1x @ M1 - W_rw1x
    c_d3 = c_rg1x @ M1 + bM1 - c_rw1x

    def aug(Wm, c):
        return np.vstack([Wm, c[None, :]]).astype(np.float32)

    def b16(a):
        return a.astype(np.float32).astype(ml_dtypes.bfloat16)

    # m-layouts: [K=256 -> (kt, p)] stored [p, kt, M]
    def klay(Wm, mcols):
        return np.ascontiguousarray(
            Wm.reshape(2, 128, mcols).transpose(1, 0, 2))

    return dict(
        # Wg0 duplicated on both partition halves so odd-batch adjx slices
        # (SBUF base partition 64) have a matching-base stationary operand.
        wg0=b16(np.vstack([f['Wg0'], f['Wg0']])),  # (128, 256)
        m0i=b16(klay(M0I, H)),                   # (128, 2, 256)
        wg1=b16(klay(f['Wg1'], H)),              # (128, 2, 256)
        m1=b16(klay(M1, DOUT)),                  # (128, 2, 64)
        wd0x=aug(-W_rg0x, c_d0),                 # (65, 256) f32
        wa1=aug(W_A1, c_d1),
        wrw0x=aug(W_rw0x, c_rw0x),
        wd2x=aug(-W_rg1x, c_d2),
        wa3=aug(W_A3, c_d3),                     # (65, 64)
        wrw1x=aug(W_rw1x, c_rw1x),
    )


# ---------------------------------------------------------------- bass program
def _build_program():
    nc = bacc.Bacc("TRN2", target_bir_lowering=False, debug=False,
                   num_devices=NCORES)
    RG = [list(range(NCORES))]

    # DRAM I/O
    d_xf = nc.dram_tensor("xf", [DIN + 1, FREE], F32R, kind="ExternalInput")
    d_xn = nc.dram_tensor("xn", [128, 4, BL * DIN], BF16, kind="ExternalInput")
    d_adjT = nc.dram_tensor("adjT", [128, 4, N], BF16, kind="ExternalInput")
    d_wg0 = nc.dram_tensor("wg0", [128, H], BF16, kind="ExternalInput")
    d_m0i = nc.dram_tensor("m0i", [128, 2, H], BF16, kind="ExternalInput")
    d_wg1 = nc.dram_tensor("wg1", [128, 2, H], BF16, kind="ExternalInput")
    d_m1 = nc.dram_tensor("m1", [128, 2, DOUT], BF16, kind="ExternalInput")
    d_wd0x = nc.dram_tensor("wd0x", [DIN + 1, H], F32R, kind="ExternalInput")
    d_wa1 = nc.dram_tensor("wa1", [DIN + 1, H], F32R, kind="ExternalInput")
    d_wrw0x = nc.dram_tensor("wrw0x", [DIN + 1, H], F32R, kind="ExternalInput")
    d_wd2x = nc.dram_tensor("wd2x", [DIN + 1, H], F32R, kind="ExternalInput")
    d_wa3 = nc.dram_tensor("wa3", [DIN + 1, DOUT], F32R, kind="ExternalInput")
    d_wrw1x = nc.dram_tensor("wrw1x", [DIN + 1, DOUT], F32R, kind="ExternalInput")
    d_ident = nc.dram_tensor("ident", [128, 128], BF16, kind="ExternalInput")
    d_out = nc.dram_tensor("outf", [DOUT, FREE], F32, kind="ExternalOutput")

    with tile.TileContext(nc) as tc:
        with (
            tc.tile_pool(name="consts", bufs=1) as consts,
            tc.tile_pool(name="xfp", bufs=1) as xfp,
            tc.tile_pool(name="inp", bufs=1) as inpool,
            tc.tile_pool(name="big", bufs=4) as bigp,
            tc.tile_pool(name="small", bufs=1) as smallp,
            tc.tile_pool(name="outp", bufs=1) as outp,
            tc.tile_pool(name="ps", bufs=4, space="PSUM") as psp,
            tc.tile_pool(name="dramp", bufs=1, space="DRAM") as dramp,
        ):
            # ---- load inputs
            xf = xfp.tile([DIN + 1, FREE], F32R, name="xf_t")
            nc.sync.dma_start(xf[:], d_xf[:])
            xn = inpool.tile([128, 4, BL * DIN], BF16, name="xn_t")
            nc.sync.dma_start(xn[:], d_xn[:])
            adjT = inpool.tile([128, 4, N], BF16, name="adjT_t")
            nc.sync.dma_start(adjT[:], d_adjT[:])

            wg0 = consts.tile([128, H], BF16, name="wg0_t")
            nc.sync.dma_start(wg0[:], d_wg0[:])
            m0i = consts.tile([128, 2, H], BF16, name="m0i_t")
            nc.sync.dma_start(m0i[:], d_m0i[:])
            wg1 = consts.tile([128, 2, H], BF16, name="wg1_t")
            nc.sync.dma_start(wg1[:], d_wg1[:])
            m1 = consts.tile([128, 2, DOUT], BF16, name="m1_t")
            nc.sync.dma_start(m1[:], d_m1[:])
            wd0x = consts.tile([DIN + 1, H], F32R, name="wd0x_t")
            nc.sync.dma_start(wd0x[:], d_wd0x[:])
            wa1 = consts.tile([DIN + 1, H], F32R, name="wa1_t")
            nc.sync.dma_start(wa1[:], d_wa1[:])
            wrw0x = consts.tile([DIN + 1, H], F32R, name="wrw0x_t")
            nc.sync.dma_start(wrw0x[:], d_wrw0x[:])
            wd2x = consts.tile([DIN + 1, H], F32R, name="wd2x_t")
            nc.sync.dma_start(wd2x[:], d_wd2x[:])
            wa3 = consts.tile([DIN + 1, DOUT], F32R, name="wa3_t")
            nc.sync.dma_start(wa3[:], d_wa3[:])
            wrw1x = consts.tile([DIN + 1, DOUT], F32R, name="wrw1x_t")
            nc.sync.dma_start(wrw1x[:], d_wrw1x[:])
            ident = consts.tile([128, 128], BF16, name="ident_t")
            nc.sync.dma_start(ident[:], d_ident[:])

            m0i_s = consts.tile([128, 2, H], BF16, name="m0i_s_t")
            m1_s = consts.tile([128, 2, DOUT], BF16, name="m1_s_t")

            # small stats / theta tiles
            stats = [smallp.tile([128, 16], F32, name=f"stats{i}_t")
                     for i in range(4)]
            ssum = [smallp.tile([128, 1], F32, name=f"ssum{i}_t")
                    for i in range(4)]
            sred = [smallp.tile([128, 1], F32, name=f"sred{i}_t")
                    for i in range(4)]
            stot = [smallp.tile([128, 1], F32, name=f"stot{i}_t")
                    for i in range(4)]
            th = [smallp.tile([128, 1], F32, name=f"th{i}_t")
                  for i in range(4)]
            cc_in = [dramp.tile([128, 1], F32, name=f"ccin{i}_t")
                     for i in range(4)]
            cc_out = [dramp.tile([128, 1], F32, name=f"ccout{i}_t",
                                 addr_space="Shared") for i in range(4)]

            def theta(i, n_cols, cnt, part=128):
                """stats[i][:, :n_cols] -> AllReduce -> th[i] (sigmoid)."""
                nc.vector.tensor_reduce(
                    ssum[i][:part, :], stats[i][:part, :n_cols],
                    axis=mybir.AxisListType.X, op=OP.add)
                nc.sync.dma_start(cc_in[i][:part, :], ssum[i][:part, :])
                nc.gpsimd.collective_compute(
                    "AllReduce", OP.add, replica_groups=RG,
                    ins=[cc_in[i][:part, :].opt()],
                    outs=[cc_out[i][:part, :].opt()])
                nc.sync.dma_start(sred[i][:part, :], cc_out[i][:part, :])
                nc.gpsimd.partition_all_reduce(
                    stot[i][:part, :], sred[i][:part, :], channels=part,
                    reduce_op=bass_isa.ReduceOp.add)
                nc.scalar.activation(th[i][:part, :], stot[i][:part, :],
                                     AF.Sigmoid, scale=1.0 / cnt)

            # ---- adj0: adjx[(2b x 64f), bp, i] = (adj @ x), bf16
            # out[(b,f),i] = sum_j x[(b,f),j]*adjT[j,i]; lhsT = xn-slice
            # [j,(b,f)-chunk] is the stationary operand, rhs = adjT[j,:].
            adjx = bigp.tile([128, 8, N], BF16, name="adjx_t", tag="big")
            for t2 in range(4):
                ps = psp.tile([128, 1024], F32, name="ps_adj0", tag="ps")
                for q in range(2):
                    bp = t2 * 2 + q
                    for jt in range(4):
                        nc.tensor.matmul(
                            ps[:, q * 512:(q + 1) * 512],
                            xn[:, jt, bp * 128:(bp + 1) * 128],
                            adjT[:, jt, :],
                            start=(jt == 0), stop=(jt == 3))
                nc.vector.tensor_copy(
                    adjx[:, t2 * 2:(t2 + 1) * 2, :],
                    ps.rearrange("p (k f) -> p k f", k=2))

            # ---- d0 = adjx@Wg0 - x@W_rg0x (+c)
            d0 = bigp.tile([128, 2 * FREE], BF16, name="d0_t", tag="big")
            d0v = d0.rearrange("p (k f) -> p k f", k=2)
            scr0 = bigp.tile([128, 2 * FREE], BF16, name="scr0_t", tag="big")
            for mt in range(2):
                msl = slice(mt * 128, (mt + 1) * 128)
                for b2 in range(8):
                    ps = psp.tile([128, 1024], F32, name="ps_d0", tag="ps")
                    for q in range(2):
                        b = b2 * 2 + q
                        qsl = slice(q * 512, (q + 1) * 512)
                        po = (b % 2) * 64
                        nc.tensor.matmul(
                            ps[:, qsl], wg0[po:po + 64, msl],
                            adjx[po:po + 64, b // 2, :],
                            start=True, stop=False)
                        nc.tensor.matmul(
                            ps[:, qsl], wd0x[:, msl],
                            xf[:, b * 512:(b + 1) * 512],
                            start=False, stop=True)
                    csl = slice(b2 * 1024, (b2 + 1) * 1024)
                    nc.vector.tensor_copy(d0v[:, mt, csl], ps[:])
                    nc.scalar.activation(
                        scr0[:, csl], d0v[:, mt, csl], AF.Square,
                        accum_out=stats[0][:, mt * 8 + b2: mt * 8 + b2 + 1])
            theta(0, 16, CNT_H)

            # th0-scaled wavelet matrix
            for kt in range(2):
                nc.scalar.activation(m0i_s[:, kt, :], m0i[:, kt, :], AF.Copy,
                                     scale=th[0][:])

            # ---- d1 = x@W_A1 + d0@(th0*M0I) (+c)
            d1 = bigp.tile([128, 2 * FREE], BF16, name="d1_t", tag="big")
            d1v = d1.rearrange("p (k f) -> p k f", k=2)
            for mt in range(2):
                msl = slice(mt * 128, (mt + 1) * 128)
                for b2 in range(8):
                    ps = psp.tile([128, 1024], F32, name="ps_d1", tag="ps")
                    for q in range(2):
                        b = b2 * 2 + q
                        qsl = slice(q * 512, (q + 1) * 512)
                        bsl = slice(b * 512, (b + 1) * 512)
                        nc.tensor.matmul(ps[:, qsl], wa1[:, msl],
                                         xf[:, bsl], start=True, stop=False)
                        for kt in range(2):
                            nc.tensor.matmul(ps[:, qsl], m0i_s[:, kt, msl],
                                             d0v[:, kt, bsl],
                                             start=False, stop=(kt == 1))
                    csl = slice(b2 * 1024, (b2 + 1) * 1024)
                    nc.vector.tensor_copy(d1v[:, mt, csl], ps[:])
                    nc.scalar.activation(
                        scr0[:, csl], d1v[:, mt, csl], AF.Square,
                        accum_out=stats[1][:, mt * 8 + b2: mt * 8 + b2 + 1])
            theta(1, 16, CNT_H)

            # ---- xw' = x@W_rw0x + th1*d1 (+c)
            xwpF = bigp.tile([128, 2 * FREE], BF16, name="xwpF_t", tag="big")
            xwpFv = xwpF.rearrange("p (k f) -> p k f", k=2)
            for mt in range(2):
                msl = slice(mt * 128, (mt + 1) * 128)
                for b2 in range(8):
                    ps = psp.tile([128, 1024], F32, name="ps_xwp", tag="ps")
                    for q in range(2):
                        b = b2 * 2 + q
                        nc.tensor.matmul(ps[:, q * 512:(q + 1) * 512],
                                         wrw0x[:, msl],
                                         xf[:, b * 512:(b + 1) * 512],
                                         start=True, stop=True)
                    csl = slice(b2 * 1024, (b2 + 1) * 1024)
                    nc.vector.scalar_tensor_tensor(
                        xwpFv[:, mt, csl], d1v[:, mt, csl], th[1][:], ps[:],
                        op0=OP.mult, op1=OP.add)

            # ---- T2: transpose xw' to node-major lhsT blocks
            # xwpN[p=j, jt, c=(b,kt), m=h-within-half]
            xwpN = bigp.tile([128, 2 * FREE], BF16, name="xwpN_t", tag="big")
            xwpNv = xwpN.rearrange("p (t c m) -> p t c m", t=4, c=32)
            for b in range(BL):
                for kt in range(2):
                    pst = psp.tile([128, 512], BF16, name="ps_t2", tag="ps")
                    for jt in range(4):
                        nc.tensor.transpose(
                            pst[:, jt * 128:(jt + 1) * 128],
                            xwpFv[:, kt, b * 512 + jt * 128:
                                  b * 512 + (jt + 1) * 128],
                            ident[:])
                    nc.vector.tensor_copy(
                        xwpNv[:, :, b * 2 + kt, :],
                        pst.rearrange("p (t m) -> p t m", t=4))

            # ---- adj1: adjw = adj @ xw'   (direct F-layout output)
            adjw = bigp.tile([128, 2 * FREE], BF16, name="adjw_t", tag="big")
            adjwv = adjw.rearrange("p (k f) -> p k f", k=2)
            for c2 in range(16):
                ps = psp.tile([128, 1024], F32, name="ps_adj1", tag="ps")
                for q in range(2):
                    c = c2 * 2 + q       # c = b*2 + kt, so q == kt, b == c2
                    for jt in range(4):
                        nc.tensor.matmul(ps[:, q * 512:(q + 1) * 512],
                                         xwpNv[:, jt, c, :],
                                         adjT[:, jt, :],
                                         start=(jt == 0), stop=(jt == 3))
                nc.vector.tensor_copy(
                    adjwv[:, :, c2 * 512:(c2 + 1) * 512],
                    ps.rearrange("p (k f) -> p k f", k=2))

            # ---- d2 = adjw@Wg1 - x@W_rg1x + xw' (+c)
            d2 = bigp.tile([128, 2 * FREE], BF16, name="d2_t", tag="big")
            d2v = d2.rearrange("p (k f) -> p k f", k=2)
            scr2 = bigp.tile([128, 2 * FREE], BF16, name="scr2_t", tag="big")
            for mt in range(2):
                msl = slice(mt * 128, (mt + 1) * 128)
                for b2 in range(8):
                    ps = psp.tile([128, 1024], F32, name="ps_d2", tag="ps")
                    for q in range(2):
                        b = b2 * 2 + q
                        qsl = slice(q * 512, (q + 1) * 512)
                        bsl = slice(b * 512, (b + 1) * 512)
                        nc.tensor.matmul(ps[:, qsl], wd2x[:, msl],
                                         xf[:, bsl], start=True, stop=False)
                        for kt in range(2):
                            nc.tensor.matmul(ps[:, qsl], wg1[:, kt, msl],
                                             adjwv[:, kt, bsl],
                                             start=False, stop=(kt == 1))
                    csl = slice(b2 * 1024, (b2 + 1) * 1024)
                    nc.vector.scalar_tensor_tensor(
                        d2v[:, mt, csl], ps[:], 1.0, xwpFv[:, mt, csl],
                        op0=OP.mult, op1=OP.add)
                    nc.scalar.activation(
                        scr2[:, csl], d2v[:, mt, csl], AF.Square,
                        accum_out=stats[2][:, mt * 8 + b2: mt * 8 + b2 + 1])
            theta(2, 16, CNT_H)

            for kt in range(2):
                nc.scalar.activation(m1_s[:, kt, :], m1[:, kt, :], AF.Copy,
                                     scale=th[2][:])

            # ---- d3 = x@W_A3 + d2@(th2*M1) (+c)   [64-row tensor]
            d3 = bigp.tile([128, 2 * FREE], BF16, name="d3_t", tag="big")
            for b2 in range(8):
                ps = psp.tile([64, 1024], F32, name="ps_d3", tag="ps")
                for q in range(2):
                    b = b2 * 2 + q
                    qsl = slice(q * 512, (q + 1) * 512)
                    bsl = slice(b * 512, (b + 1) * 512)
                    nc.tensor.matmul(ps[:, qsl], wa3[:, :], xf[:, bsl],
                                     start=True, stop=False)
                    for kt in range(2):
                        nc.tensor.matmul(ps[:, qsl], m1_s[:, kt, :],
                                         d2v[:, kt, bsl],
                                         start=False, stop=(kt == 1))
                csl = slice(b2 * 1024, (b2 + 1) * 1024)
                nc.vector.tensor_copy(d3[0:64, csl], ps[:])
                nc.scalar.activation(
                    scr2[0:64, csl], d3[0:64, csl], AF.Square,
                    accum_out=stats[3][0:64, b2:b2 + 1])
            theta(3, 8, CNT_O, part=64)

            # ---- out = x@W_rw1x + th3*d3 (+c)
            for b2 in range(8):
                ps = psp.tile([64, 1024], F32, name="ps_out", tag="ps")
                for q in range(2):
                    b = b2 * 2 + q
                    nc.tensor.matmul(ps[:, q * 512:(q + 1) * 512],
                                     wrw1x[:, :],
                                     xf[:, b * 512:(b + 1) * 512],
                                     start=True, stop=True)
                csl = slice(b2 * 1024, (b2 + 1) * 1024)
                outf = outp.tile([DOUT, 1024], F32, name="outf_t", tag="outf",
                                 bufs=2)
                nc.vector.scalar_tensor_tensor(
                    outf[:], d3[0:64, csl], th[3][0:64, :], ps[:],
                    op0=OP.mult, op1=OP.add)
                nc.sync.dma_start(d_out[:, csl], outf[:])

    nc.compile()
    return nc


def _get_compiled():
    global _COMPILED
    if _COMPILED is None:
        _COMPILED = _build_program()
    return _COMPILED


# ---------------------------------------------------------------- entry point
def _make_in_maps(inputs):
    x = np.asarray(inputs['x'], np.float32)              # (B, N, 64)
    w = _fold_weights(inputs)

    # global host-side layouts
    #   xf[f, b*N+n] = x[b,n,f]; row 64 = 1.0
    xT = np.ascontiguousarray(x.transpose(2, 0, 1))      # (64, B, N) f32
    adj = np.asarray(inputs['adj'], np.float32)
    # adjT layout [p, jt, i] with j = jt*128+p
    adjT = np.ascontiguousarray(
        adj.T.reshape(4, 128, N).transpose(1, 0, 2)).astype(ml_dtypes.bfloat16)

    # xn layout [p, jt, b*64+f] with n = jt*128+p (per core below)
    x_bf = x.astype(ml_dtypes.bfloat16)

    ident = np.eye(128, dtype=ml_dtypes.bfloat16)

    in_maps = []
    for c in range(NCORES):
        bsl = slice(c * BL, (c + 1) * BL)
        xf_c = np.empty((DIN + 1, FREE), np.float32)
        xf_c[:DIN] = xT[:, bsl, :].reshape(DIN, FREE)
        xf_c[DIN] = 1.0
        # xn[p, jt, b*64+f] = x[c*BL+b, jt*128+p, f]
        xn_c = np.ascontiguousarray(
            x_bf[bsl].reshape(BL, 4, 128, DIN).transpose(2, 1, 0, 3)
            .reshape(128, 4, BL * DIN))
        in_maps.append(dict(
            xf=xf_c, xn=xn_c, adjT=adjT,
            wg0=w['wg0'], m0i=w['m0i'], wg1=w['wg1'], m1=w['m1'],
            wd0x=w['wd0x'], wa1=w['wa1'], wrw0x=w['wrw0x'],
            wd2x=w['wd2x'], wa3=w['wa3'], wrw1x=w['wrw1x'],
            ident=ident,
        ))
    return in_maps


def _run(nc, in_maps, trace=False):
    return run_bass_kernel_spmd(nc, in_maps, core_ids=list(range(NCORES)),
                                trace=trace)


def _gather(res):
    out = np.empty((B, N, DOUT), np.float32)
    for c in range(NCORES):
        of = np.asarray(res.results[c]['outf']).reshape(DOUT, BL, N)
        out[c * BL:(c + 1) * BL] = of.transpose(1, 2, 0)
    return out


def kernel(**inputs):
    in_maps = _make_in_maps(inputs)
    nc = _get_compiled()
    return _gather(_run(nc, in_maps))


if __name__ == '__main__':
    import reference
    inp = reference.setup_inputs()
    inp = {k: np.asarray(v) for k, v in inp.items()}
    got = kernel(**inp)
    exp = np.asarray(reference.reference(**reference.setup_inputs()))
    rel = np.linalg.norm(got - exp) / np.linalg.norm(exp)
    print('rel l2 err:', rel)



# revision 11
# speedup vs baseline: 1.3795x; 1.3795x over previous
"""Trainium2 Bass kernel for nn_GawKAN (2-layer graph-wavelet net with 4
global-MSE sigmoid mix gates).

Data-parallel over batch B=128 across 8 NeuronCores (16 batches/core);
adj + weights replicated. Each theta_i = sigmoid(mean(d_i^2)) needs one
8-core AllReduce of a [128,1] partial-sum vector.

v2 restructure: all theta-dependence is reduced to scalar coefficients on
theta-FREE tensor bases so the PE never idles across AllReduce latency:

  adjx  = adj @ x,  adj2x = (adj@adj) @ x          (adj^2 folded on host)
  d0 = adjx@Wg0 + x@(-W_rg0x) (+c)                  -> theta0
  d1 = x@W_A1 + d0@(th0*M0I) (+c)                   -> theta1
  d2 = Z_A + th1*ZB                                 -> theta2
     Z_A = adjx@G0a + x@[W_rw0x-W_rg1x] (+c,+r)        (theta-free)
     ZB  = adj2x@(th0*K2a) + adjx@(A1g+th0*(K1-K2b))
           + x@(W_A1-th0*WK2) (+c,+r)                  (needs th0 only)
  d3 = x@W_A3 (+c) + th2*(d2@M1)                    -> theta3
  out = x@W_rw1x (+c) + th3*d3

r-terms (adj applied to constants, rank-1 in node space) ride in an
augmented x of 66 rows: [x(64); ones; r], r = adj.sum(1) tiled per batch.
Everything bf16 on the PE with fp32 PSUM accumulation; a dummy AllReduce
at program start absorbs the collective-stream warmup.
"""
import numpy as np
import ml_dtypes

import concourse.bass as bass
import concourse.bacc as bacc
import concourse.tile as tile
import concourse.mybir as mybir
from concourse import bass_isa
from concourse.bass_utils import run_bass_kernel_spmd

NCORES = 8
B, N, DIN, H, DOUT = 128, 512, 64, 256, 64
BL = B // NCORES                 # 16 batches per core
FREE = BL * N                    # 8192 free positions per core
MAX_LEVEL, FULL_LEN = 3, 8
XR = DIN + 2                     # x rows: 64 features + ones + r

F32 = mybir.dt.float32
F32R = mybir.dt.float32r
BF16 = mybir.dt.bfloat16
AF = mybir.ActivationFunctionType
OP = mybir.AluOpType

CNT_H = float(B * N * H)
CNT_O = float(B * N * DOUT)

_COMPILED = None


# ---------------------------------------------------------------- host algebra
def _make_patterns():
    pats = []
    for j in range(1, MAX_LEVEL + 1):
        vec = [2.0 ** (-j / 2.0)]
        for k in range(1, j + 1):
            seg = np.zeros(2 ** (j - k))
            seg[0] = 2.0 ** (-k / 2.0)
            vec.extend(seg.tolist())
        full = np.zeros(FULL_LEN, dtype=np.float64)
        arr = np.asarray(vec, dtype=np.float64)
        full[: arr.shape[0]] = arr
        pats.append(full)
    return np.stack(pats)


def _gumbel_coeff(logits, gumb, Wnl):
    P = _make_patterns()
    a = Wnl.astype(np.float64) @ P.T
    z = logits[None, :].astype(np.float64) + gumb.astype(np.float64)
    idx = np.argmax(z, axis=-1)
    return a[np.arange(a.shape[0]), idx]


def _fold_weights(inp):
    f = {k: np.asarray(v, np.float64) for k, v in inp.items()}
    coeff0 = _gumbel_coeff(f['logits0'], f['gumb0'], f['Wnl0'])
    coeff1 = _gumbel_coeff(f['logits1'], f['gumb1'], f['Wnl1'])

    M0I = coeff0[:, None] * f['Wwo0'] + np.eye(H)
    bwo0e = f['bnl0'] @ f['Wwo0'] + f['bwo0']
    M1 = coeff1[:, None] * f['Wwo1']
    bM1 = f['bnl1'] @ f['Wwo1'] + f['bwo1']

    W_rg0x = f['Wres'] @ f['Wrg0']; c_rg0x = f['bres'] @ f['Wrg0'] + f['brg0']
    W_rw0x = f['Wres'] @ f['Wrw0']; c_rw0x = f['bres'] @ f['Wrw0'] + f['brw0']
    W_rg1x = f['Wres'] @ f['Wrg1']; c_rg1x = f['bres'] @ f['Wrg1'] + f['brg1']
    W_rw1x = f['Wres'] @ f['Wrw1']; c_rw1x = f['bres'] @ f['Wrw1'] + f['brw1']

    c_d0 = f['bg0'] - c_rg0x
    W_A1 = W_rg0x @ M0I - W_rw0x
    c_d1 = c_rg0x @ M0I + bwo0e - c_rw0x
    c_d2 = f['bg1'] - c_rg1x
    W_A3 = W_rg1x @ M1 - W_rw1x
    c_d3 = c_rg1x @ M1 + bM1 - c_rw1x

    Wg1 = f['Wg1']
    K1 = f['Wg0'] @ M0I                 # V's adjx weight
    WK2 = W_rg0x @ M0I                  # V's x weight (negated)
    cV0 = c_d0 @ M0I                    # V's constant
    G0a = W_rw0x @ Wg1
    g0r = c_rw0x @ Wg1
    A1g = W_A1 @ Wg1
    a1r = c_d1 @ Wg1
    K2a = K1 @ Wg1
    K2b = WK2 @ Wg1
    vr = cV0 @ Wg1

    def b16(a):
        return np.ascontiguousarray(a).astype(np.float32).astype(
            ml_dtypes.bfloat16)

    def aug(Wm, c, rr=None):
        rows = [Wm, np.asarray(c)[None, :]]
        rows.append(np.zeros_like(rows[1]) if rr is None
                    else np.asarray(rr)[None, :])
        return b16(np.vstack(rows))                     # (66, M)

    def dup(Wm):
        return b16(np.vstack([Wm, Wm]))                 # (128, M)

    def klay(Wm, mcols):
        return b16(np.ascontiguousarray(
            Wm.reshape(2, 128, mcols).transpose(1, 0, 2)))

    return dict(
        wg0d=dup(f['Wg0']),                             # (128, 256)
        wd0x=aug(-W_rg0x, c_d0),                        # (66, 256)
        m0i=klay(M0I, H),                               # (128, 2, 256)
        wa1=aug(W_A1, c_d1),                            # (66, 256)
        za_a=dup(G0a),                                  # (128, 256)
        za_x=aug(W_rw0x - W_rg1x, c_rw0x + c_d2, g0r),  # (66, 256)
        zb_a0=dup(A1g),
        zb_a1=dup(K1 - K2b),
        zb_x0=aug(W_A1, c_d1, a1r),
        zb_x1=aug(-WK2, cV0, vr),
        zb_b1=dup(K2a),
        m1=klay(M1, DOUT),                              # (128, 2, 64)
        wa3=aug(W_A3, c_d3),                            # (66, 64)
        wrw1x=aug(W_rw1x, c_rw1x),                      # (66, 64)
    )


# ---------------------------------------------------------------- bass program
def _build_program():
    nc = bacc.Bacc("TRN2", target_bir_lowering=False, debug=False,
                   num_devices=NCORES)
    RG = [list(range(NCORES))]

    d_xf = nc.dram_tensor("xf", [XR, FREE], BF16, kind="ExternalInput")
    d_xn = nc.dram_tensor("xn", [128, 4, BL * DIN], BF16, kind="ExternalInput")
    d_adjT = nc.dram_tensor("adjT", [128, 4, N], BF16, kind="ExternalInput")
    d_adj2T = nc.dram_tensor("adj2T", [128, 4, N], BF16, kind="ExternalInput")
    d_w = {k: nc.dram_tensor(k, list(shp), BF16, kind="ExternalInput")
           for k, shp in _FOLD_SHAPES.items()}
    d_out = nc.dram_tensor("outf", [DOUT, FREE], F32, kind="ExternalOutput")

    with tile.TileContext(nc) as tc:
        with (
            tc.tile_pool(name="consts", bufs=1) as consts,
            tc.tile_pool(name="inp", bufs=1) as inpool,
            tc.tile_pool(name="mid", bufs=4) as midp,
            tc.tile_pool(name="big", bufs=2) as bigp,
            tc.tile_pool(name="small", bufs=1) as smallp,
            tc.tile_pool(name="outp", bufs=2) as outp,
            tc.tile_pool(name="ps", bufs=3, space="PSUM") as psp,
            tc.tile_pool(name="dramp", bufs=1, space="DRAM") as dramp,
        ):
            # ---- theta plumbing tiles
            stats = [smallp.tile([128, 16], F32, name=f"stats{i}_t")
                     for i in range(4)]
            ssum = [smallp.tile([128, 1], F32, name=f"ssum{i}_t")
                    for i in range(4)]
            sredf = [smallp.tile([128, 1], F32, name=f"sredf{i}_t")
                     for i in range(4)]
            stot = [smallp.tile([128, 1], F32, name=f"stot{i}_t")
                    for i in range(4)]
            th = [smallp.tile([128, 1], F32, name=f"th{i}_t")
                  for i in range(4)]
            dumb = smallp.tile([128, 1], F32, name="dumb_t")
            cc_in = [dramp.tile([128, 1], F32, name=f"ccin{i}_t")
                     for i in range(5)]
            cc_out = [dramp.tile([128, 1], F32, name=f"ccout{i}_t",
                                 addr_space="Shared") for i in range(5)]

            # ---- dummy AllReduce: warm the collective stream early
            nc.gpsimd.memset(dumb[:], 0.0)
            nc.sync.dma_start(cc_in[4][:], dumb[:])
            nc.gpsimd.collective_compute(
                "AllReduce", OP.add, replica_groups=RG,
                ins=[cc_in[4][:].opt()], outs=[cc_out[4][:].opt()])

            # ---- input DMAs (adj0 gates first: xn + adjT first)
            xn = midp.tile([128, 4, BL * DIN], BF16, name="xn_t", tag="mid")
            nc.sync.dma_start(xn[:], d_xn[:])
            adjT = midp.tile([128, 4, N], BF16, name="adjT_t", tag="mid")
            nc.sync.dma_start(adjT[:], d_adjT[:])
            adj2T = midp.tile([128, 4, N], BF16, name="adj2T_t", tag="mid")
            nc.sync.dma_start(adj2T[:], d_adj2T[:])
            xf = inpool.tile([XR, FREE], BF16, name="xf_t")
            nc.sync.dma_start(xf[:], d_xf[:])

            W = {}
            for k, shp in _FOLD_SHAPES.items():
                W[k] = consts.tile(list(shp), BF16, name=f"{k}_t")
                nc.sync.dma_start(W[k][:], d_w[k][:])

            # runtime-combined stationaries
            m0i_s = consts.tile([128, 2, H], BF16, name="m0i_s_t")
            zb_x = consts.tile([XR, H], BF16, name="zb_x_t")
            zb_a = consts.tile([128, H], BF16, name="zb_a_t")
            zb_b = consts.tile([128, H], BF16, name="zb_b_t")

            scr = smallp.tile([128, 1024], BF16, name="scr_t")

            def theta(i, n_cols, cnt, part=128):
                """stats[i][:, :n_cols] -> AllReduce -> PE partition-sum ->
                sigmoid -> th[i]."""
                nc.vector.tensor_reduce(
                    ssum[i][:part, :], stats[i][:part, :n_cols],
                    axis=mybir.AxisListType.X, op=OP.add)
                nc.sync.dma_start(cc_in[i][:part, :], ssum[i][:part, :])
                nc.gpsimd.collective_compute(
                    "AllReduce", OP.add, replica_groups=RG,
                    ins=[cc_in[i][:part, :].opt()],
                    outs=[cc_out[i][:part, :].opt()])
                nc.sync.dma_start(sredf[i][:part, :], cc_out[i][:part, :])

            def theta_fin(i, cnt, part=128):
                """Partition reduction + sigmoid (issue after the matmuls
                that fill this AR window; gpsimd is otherwise idle here)."""
                nc.gpsimd.partition_all_reduce(
                    stot[i][:part, :], sredf[i][:part, :], channels=part,
                    reduce_op=bass_isa.ReduceOp.add)
                nc.scalar.activation(th[i][:part, :], stot[i][:part, :],
                                     AF.Sigmoid, scale=1.0 / cnt)

            # ---- adjx / adj2x (shared stationary xn chunks)
            adjx = inpool.tile([128, 8, N], BF16, name="adjx_t")
            adj2x = inpool.tile([128, 8, N], BF16, name="adj2x_t")
            for t2 in range(4):
                ps1 = psp.tile([128, 1024], F32, name="ps_adj0", tag="ps")
                ps2 = psp.tile([128, 1024], F32, name="ps_adj2", tag="ps")
                for q in range(2):
                    bp = t2 * 2 + q
                    qsl = slice(q * 512, (q + 1) * 512)
                    for jt in range(4):
                        st = xn[:, jt, bp * 128:(bp + 1) * 128]
                        nc.tensor.matmul(ps1[:, qsl], st, adjT[:, jt, :],
                                         start=(jt == 0), stop=(jt == 3))
                        nc.tensor.matmul(ps2[:, qsl], st, adj2T[:, jt, :],
                                         start=(jt == 0), stop=(jt == 3))
                nc.vector.tensor_copy(
                    adjx[:, t2 * 2:(t2 + 1) * 2, :],
                    ps1.rearrange("p (k f) -> p k f", k=2))
                nc.scalar.copy(
                    adj2x[:, t2 * 2:(t2 + 1) * 2, :],
                    ps2.rearrange("p (k f) -> p k f", k=2))

            # ---- d0 = adjx@Wg0 + x@(-W_rg0x) (+c)
            d0 = bigp.tile([128, 2 * FREE], BF16, name="d0_t", tag="big")
            d0v = d0.rearrange("p (k f) -> p k f", k=2)
            for mt in range(2):
                msl = slice(mt * 128, (mt + 1) * 128)
                for b2 in range(8):
                    ps = psp.tile([128, 1024], F32, name="ps_d0", tag="ps")
                    for q in range(2):
                        b = b2 * 2 + q
                        nc.tensor.matmul(
                            ps[:, q * 512:(q + 1) * 512], W['wd0x'][:, msl],
                            xf[:, b * 512:(b + 1) * 512],
                            start=True, stop=False)
                    for q in range(2):
                        b = b2 * 2 + q
                        po = q * 64
                        nc.tensor.matmul(
                            ps[:, q * 512:(q + 1) * 512],
                            W['wg0d'][po:po + 64, msl],
                            adjx[po:po + 64, b // 2, :],
                            start=False, stop=True)
                    csl = slice(b2 * 1024, (b2 + 1) * 1024)
                    nc.vector.tensor_copy(d0v[:, mt, csl], ps[:])
                    nc.scalar.activation(
                        scr[:], d0v[:, mt, csl], AF.Square,
                        accum_out=stats[0][:, mt * 8 + b2: mt * 8 + b2 + 1])
            theta(0, 16, CNT_H)

            # ---- Z_A = adjx@G0a + x@[W_rw0x - W_rg1x] (+c,+r)  [fills AR0]
            zA = bigp.tile([128, 2 * FREE], BF16, name="zA_t", tag="big")
            zAv = zA.rearrange("p (k f) -> p k f", k=2)
            for mt in range(2):
                msl = slice(mt * 128, (mt + 1) * 128)
                for b2 in range(8):
                    ps = psp.tile([128, 1024], F32, name="ps_za", tag="ps")
                    for q in range(2):
                        b = b2 * 2 + q
                        nc.tensor.matmul(
                            ps[:, q * 512:(q + 1) * 512], W['za_x'][:, msl],
                            xf[:, b * 512:(b + 1) * 512],
                            start=True, stop=False)
                    for q in range(2):
                        b = b2 * 2 + q
                        po = q * 64
                        nc.tensor.matmul(
                            ps[:, q * 512:(q + 1) * 512],
                            W['za_a'][po:po + 64, msl],
                            adjx[po:po + 64, b // 2, :],
                            start=False, stop=True)
                    csl = slice(b2 * 1024, (b2 + 1) * 1024)
                    nc.vector.tensor_copy(zAv[:, mt, csl], ps[:])
            theta_fin(0, CNT_H)

            # ---- runtime stationary combines (need th0)
            for kt in range(2):
                nc.scalar.activation(m0i_s[:, kt, :], W['m0i'][:, kt, :],
                                     AF.Copy, scale=th[0][:])
            nc.vector.scalar_tensor_tensor(
                zb_x[:], W['zb_x1'][:], th[0][0:XR, :], W['zb_x0'][:],
                op0=OP.mult, op1=OP.add)
            nc.vector.scalar_tensor_tensor(
                zb_a[:], W['zb_a1'][:], th[0][:], W['zb_a0'][:],
                op0=OP.mult, op1=OP.add)
            nc.scalar.activation(zb_b[:], W['zb_b1'][:], AF.Copy,
                                 scale=th[0][:])

            # ---- d1 = x@W_A1 + d0@(th0*M0I) (+c); stats straight off PSUM
            for mt in range(2):
                msl = slice(mt * 128, (mt + 1) * 128)
                for b2 in range(8):
                    ps = psp.tile([128, 1024], F32, name="ps_d1", tag="ps")
                    for q in range(2):
                        b = b2 * 2 + q
                        bsl = slice(b * 512, (b + 1) * 512)
                        qsl = slice(q * 512, (q + 1) * 512)
                        nc.tensor.matmul(ps[:, qsl], W['wa1'][:, msl],
                                         xf[:, bsl], start=True, stop=False)
                        for kt in range(2):
                            nc.tensor.matmul(ps[:, qsl], m0i_s[:, kt, msl],
                                             d0v[:, kt, bsl],
                                             start=False, stop=(kt == 1))
                    nc.scalar.activation(
                        scr[:], ps[:], AF.Square,
                        accum_out=stats[1][:, mt * 8 + b2: mt * 8 + b2 + 1])
            theta(1, 16, CNT_H)

            # ---- x@W_A3, x@W_rw1x (theta-free)  [fills AR1]
            a3 = midp.tile([DOUT, FREE], BF16, name="a3_t", tag="mid")
            h0 = midp.tile([DOUT, FREE], BF16, name="h0_t", tag="mid")
            for b2 in range(8):
                ps = psp.tile([64, 1024], F32, name="ps_a3", tag="ps")
                for q in range(2):
                    b = b2 * 2 + q
                    nc.tensor.matmul(ps[:, q * 512:(q + 1) * 512],
                                     W['wa3'][:, :],
                                     xf[:, b * 512:(b + 1) * 512],
                                     start=True, stop=True)
                csl = slice(b2 * 1024, (b2 + 1) * 1024)
                nc.vector.tensor_copy(a3[:, csl], ps[:])
            for b2 in range(8):
                ps = psp.tile([64, 1024], F32, name="ps_h0", tag="ps")
                for q in range(2):
                    b = b2 * 2 + q
                    nc.tensor.matmul(ps[:, q * 512:(q + 1) * 512],
                                     W['wrw1x'][:, :],
                                     xf[:, b * 512:(b + 1) * 512],
                                     start=True, stop=True)
                csl = slice(b2 * 1024, (b2 + 1) * 1024)
                nc.vector.tensor_copy(h0[:, csl], ps[:])
            theta_fin(1, CNT_H)

            # ---- ZB matmuls (need th0 only) -> d2 = Z_A + th1*ZB
            d2 = bigp.tile([128, 2 * FREE], BF16, name="d2_t", tag="big")
            d2v = d2.rearrange("p (k f) -> p k f", k=2)
            for mt in range(2):
                msl = slice(mt * 128, (mt + 1) * 128)
                for b2 in range(8):
                    ps = psp.tile([128, 1024], F32, name="ps_zb", tag="ps")
                    for q in range(2):
                        b = b2 * 2 + q
                        bsl = slice(b * 512, (b + 1) * 512)
                        qsl = slice(q * 512, (q + 1) * 512)
                        po = q * 64
                        nc.tensor.matmul(ps[:, qsl], zb_x[:, msl],
                                         xf[:, bsl], start=True, stop=False)
                        nc.tensor.matmul(ps[:, qsl], zb_a[po:po + 64, msl],
                                         adjx[po:po + 64, b // 2, :],
                                         start=False, stop=False)
                        nc.tensor.matmul(ps[:, qsl], zb_b[po:po + 64, msl],
                                         adj2x[po:po + 64, b // 2, :],
                                         start=False, stop=True)
                    csl = slice(b2 * 1024, (b2 + 1) * 1024)
                    nc.vector.scalar_tensor_tensor(
                        d2v[:, mt, csl], ps[:], th[1][:], zAv[:, mt, csl],
                        op0=OP.mult, op1=OP.add)
                    nc.scalar.activation(
                        scr[:], d2v[:, mt, csl], AF.Square,
                        accum_out=stats[2][:, mt * 8 + b2: mt * 8 + b2 + 1])
            theta(2, 16, CNT_H)

            # ---- d2@M1  [fills AR2]
            d2m1 = midp.tile([DOUT, FREE], BF16, name="d2m1_t", tag="mid")
            for b2 in range(8):
                ps = psp.tile([64, 1024], F32, name="ps_dm", tag="ps")
                for q in range(2):
                    b = b2 * 2 + q
                    bsl = slice(b * 512, (b + 1) * 512)
                    qsl = slice(q * 512, (q + 1) * 512)
                    for kt in range(2):
                        nc.tensor.matmul(ps[:, qsl], W['m1'][:, kt, :],
                                         d2v[:, kt, bsl],
                                         start=(kt == 0), stop=(kt == 1))
                csl = slice(b2 * 1024, (b2 + 1) * 1024)
                nc.scalar.copy(d2m1[:, csl], ps[:])
            theta_fin(2, CNT_H)

            # ---- d3 = a3 + th2*d2m1 ; stats -> theta3
            d3 = midp.tile([DOUT, FREE], BF16, name="d3_t", tag="mid")
            for c in range(8):
                csl = slice(c * 1024, (c + 1) * 1024)
                nc.vector.scalar_tensor_tensor(
                    d3[:, csl], d2m1[:, csl], th[2][0:64, :], a3[:, csl],
                    op0=OP.mult, op1=OP.add)
                nc.scalar.activation(
                    scr[0:64, :], d3[:, csl], AF.Square,
                    accum_out=stats[3][0:64, c:c + 1])
            theta(3, 8, CNT_O, part=64)
            theta_fin(3, CNT_O, part=64)

            # ---- out = h0 + th3*d3
            for c in range(8):
                csl = slice(c * 1024, (c + 1) * 1024)
                outf = outp.tile([DOUT, 1024], F32, name="outf_t", tag="outf")
                nc.vector.scalar_tensor_tensor(
                    outf[:], d3[:, csl], th[3][0:64, :], h0[:, csl],
                    op0=OP.mult, op1=OP.add)
                nc.sync.dma_start(d_out[:, csl], outf[:])

    nc.compile()
    return nc


_FOLD_SHAPES = {
    'wg0d': (128, H), 'wd0x': (XR, H), 'm0i': (128, 2, H), 'wa1': (XR, H),
    'za_a': (128, H), 'za_x': (XR, H), 'zb_a0': (128, H), 'zb_a1': (128, H),
    'zb_x0': (XR, H), 'zb_x1': (XR, H), 'zb_b1': (128, H),
    'm1': (128, 2, DOUT), 'wa3': (XR, DOUT), 'wrw1x': (XR, DOUT),
}


def _get_compiled():
    global _COMPILED
    if _COMPILED is None:
        _COMPILED = _build_program()
    return _COMPILED


# ---------------------------------------------------------------- entry point
def _make_in_maps(inputs):
    x = np.asarray(inputs['x'], np.float32)              # (B, N, 64)
    w = _fold_weights(inputs)

    adj = np.asarray(inputs['adj'], np.float32)
    adj2 = adj @ adj
    r = adj.sum(axis=1)

    def tlay(a):                                         # [p, jt, i], j=jt*128+p
        return np.ascontiguousarray(
            a.T.reshape(4, 128, N).transpose(1, 0, 2)).astype(
                ml_dtypes.bfloat16)

    adjT = tlay(adj)
    adj2T = tlay(adj2)
    r_rep = np.tile(r, BL).astype(ml_dtypes.bfloat16)    # (FREE,)

    xT = np.ascontiguousarray(x.transpose(2, 0, 1))      # (64, B, N) f32
    x_bf = x.astype(ml_dtypes.bfloat16)

    in_maps = []
    for c in range(NCORES):
        bsl = slice(c * BL, (c + 1) * BL)
        xf_c = np.empty((XR, FREE), ml_dtypes.bfloat16)
        xf_c[:DIN] = xT[:, bsl, :].reshape(DIN, FREE).astype(
            ml_dtypes.bfloat16)
        xf_c[DIN] = 1.0
        xf_c[DIN + 1] = r_rep
        xn_c = np.ascontiguousarray(
            x_bf[bsl].reshape(BL, 4, 128, DIN).transpose(2, 1, 0, 3)
            .reshape(128, 4, BL * DIN))
        m = dict(xf=xf_c, xn=xn_c, adjT=adjT, adj2T=adj2T)
        m.update({k: w[k] for k in _FOLD_SHAPES})
        in_maps.append(m)
    return in_maps


def _run(nc, in_maps, trace=False):
    return run_bass_kernel_spmd(nc, in_maps, core_ids=list(range(NCORES)),
                                trace=trace)


def _gather(res):
    out = np.empty((B, N, DOUT), np.float32)
    for c in range(NCORES):
        of = np.asarray(res.results[c]['outf']).reshape(DOUT, BL, N)
        out[c * BL:(c + 1) * BL] = of.transpose(1, 2, 0)
    return out


def kernel(**inputs):
    in_maps = _make_in_maps(inputs)
    nc = _get_compiled()
    return _gather(_run(nc, in_maps))


if __name__ == '__main__':
    import reference
    inp = reference.setup_inputs()
    inp = {k: np.asarray(v) for k, v in inp.items()}
    got = kernel(**inp)
    exp = np.asarray(reference.reference(**reference.setup_inputs()))
    rel = np.linalg.norm(got - exp) / np.linalg.norm(exp)
    print('rel l2 err:', rel)
